# revision 20
# baseline (speedup 1.0000x reference)
"""Local (sliding-window) self-attention Bass kernel for 8 TRN2 NeuronCores.

Problem: B=4, T=4096, C=512, H=8 heads, head_dim=64, window=15.
Sharding: 8 cores = batch(4) x seq-halves(2). Each core processes 2048 query
tokens of one batch element; its x chunk carries a 7-token halo on each side
(zero-padded at sequence edges, matching the reference's jnp.pad semantics),
padded to 2080 rows for 128/32 alignment.

Per-core dataflow (bf16 matmuls, fp32 PSUM accumulation):
  x chunk --mask*cast--> xb bf16 --one batched XBAR DMA transpose/tile--> xT_all
  qT = Wq-stationary matmuls + bias (feature-major, per head-pair tiles)
  kT = Wkv[:, :C]-stationary matmuls + bias (feature-major)
  v_tok = xT-stationary matmuls + bias (token-major)
  attention, software-pipelined over (block, head-pair) units:
    scores for both heads of a pair -> one [128,320] PSUM tile
    one exp activation (3D AP) -> al ring [128,512] with zeroed gaps
    per head on DVE: band-mask+den accum (STT), recip (paired), normalize (TS)
    alpha transpose on the PE array (4x [128,128] -> one [128,512] bf16 PSUM)
    one scalar copy -> alT SBUF; AV matmuls (K=128 incl. zero tail rows)
  out = attnT-stationary proj matmuls, (ps + bproj) * mask on DVE, DMA out
"""

import math
from contextlib import ExitStack

import ml_dtypes
import numpy as np

import concourse.bacc as bacc
import concourse.bass as bass
import concourse.mybir as mybir
import concourse.tile as tile
from concourse import bass_utils
from concourse.masks import make_identity

B, T, C, H, WIN = 4, 4096, 512, 8, 15
D = C // H            # 64
PAD = WIN // 2        # 7
NTOK = T // 2         # 2048 query tokens per core
NKV = 2080            # kv rows per core: 7 + 2048 + 7 = 2062, padded to 2080
NB = NTOK // 128      # 16 query blocks
KCH = [512, 512, 512, 512, 32]  # kv token chunks for feature-major matmuls
SCALE = math.log(WIN) / D
F32 = mybir.dt.float32
BF16 = mybir.dt.bfloat16
MULT = mybir.AluOpType.mult


def _band_mask() -> np.ndarray:
    """[128,160] multiplicative band: band[p, j] = 1 iff p <= j <= p+14."""
    p = np.arange(128)[:, None]
    j = np.arange(160)[None, :]
    return ((j >= p) & (j <= p + WIN - 1)).astype(ml_dtypes.bfloat16)


def build_program() -> bacc.Bacc:
    nc = bacc.Bacc("TRN2", target_bir_lowering=False, debug=False,
                   enable_asserts=False, num_devices=8)

    xd = nc.dram_tensor("x", [NKV, C], F32, kind="ExternalInput").ap()
    maskd = nc.dram_tensor("mask", [NKV], F32, kind="ExternalInput").ap()
    wqd = nc.dram_tensor("wq", [C, C], F32, kind="ExternalInput").ap()
    bqd = nc.dram_tensor("bq", [C], F32, kind="ExternalInput").ap()
    wkvd = nc.dram_tensor("wkv", [C, 2 * C], F32, kind="ExternalInput").ap()
    bkvd = nc.dram_tensor("bkv", [2 * C], F32, kind="ExternalInput").ap()
    wpd = nc.dram_tensor("wproj", [C, C], F32, kind="ExternalInput").ap()
    bpd = nc.dram_tensor("bproj", [C], F32, kind="ExternalInput").ap()
    bandd = nc.dram_tensor("band", [128, 160], BF16, kind="ExternalInput").ap()
    outd = nc.dram_tensor("out", [NTOK, C], F32, kind="ExternalOutput").ap()

    with tile.TileContext(nc) as tc, ExitStack() as ctx:
        sb = ctx.enter_context(tc.tile_pool(name="sb", bufs=1))
        sb_x = ctx.enter_context(tc.tile_pool(name="sb_x", bufs=3))
        sb_xin = ctx.enter_context(tc.tile_pool(name="sb_xin", bufs=8))
        sb_alt = ctx.enter_context(tc.tile_pool(name="sb_alt", bufs=3))
        sb_den = ctx.enter_context(tc.tile_pool(name="sb_den", bufs=4))
        sb_o = ctx.enter_context(tc.tile_pool(name="sb_o", bufs=3))
        pp_big = ctx.enter_context(tc.tile_pool(name="pp_big", bufs=1, space="PSUM"))
        pp_sc = ctx.enter_context(tc.tile_pool(name="pp_sc", bufs=2, space="PSUM"))
        pp_tp = ctx.enter_context(tc.tile_pool(name="pp_tp", bufs=4, space="PSUM"))
        pp_at = ctx.enter_context(tc.tile_pool(name="pp_at", bufs=1, space="PSUM"))

        # ---- persistent SBUF tensors ----
        xT = sb.tile([128, 4 * NKV], BF16, tag="xT", name="xT")
        qT = [sb.tile([128, NTOK], BF16, tag=f"qT{i}", name=f"qT{i}") for i in range(4)]
        kT = [sb.tile([128, NKV], BF16, tag=f"kT{i}", name=f"kT{i}") for i in range(4)]
        v_tok = [sb.tile([128, C], BF16, tag=f"vtok{i}", name=f"vtok{i}") for i in range(17)]
        aT = sb.tile([128, 4 * NTOK], BF16, tag="aT", name="aT")
        band = sb.tile([128, 160], BF16, tag="band")
        ident = sb.tile([128, 128], BF16, tag="ident")
        wq = [sb.tile([128, C], BF16, tag=f"wq{i}", name=f"wq{i}") for i in range(4)]
        wk = [sb.tile([128, C], BF16, tag=f"wk{i}", name=f"wk{i}") for i in range(4)]
        wv = [sb.tile([128, C], BF16, tag=f"wv{i}", name=f"wv{i}") for i in range(4)]
        wp = [sb.tile([128, C], BF16, tag=f"wp{i}", name=f"wp{i}") for i in range(4)]
        bq_t = sb.tile([128, 4], F32, tag="bq")       # per-partition q bias
        bk_t = sb.tile([128, 4], F32, tag="bk")       # per-partition k bias
        bvB = sb.tile([128, C], F32, tag="bvB")       # v bias bcast over partitions
        bpB = sb.tile([128, C], F32, tag="bpB")       # proj bias bcast
        mq = sb.tile([128, NB], F32, tag="mq")        # query-token mask, per block
        mask_t = sb.tile([128, 17], F32, tag="mask_t")  # mask, token-tiled
        ones_r = sb.tile([1, 128], BF16, tag="ones_r")
        bp_row = sb.tile([1, C], BF16, tag="bp_row")
        xf_all = sb.tile([128, 17 * C], F32, tag="xf_all", name="xf_all")
        xb_all = sb.tile([128, 17 * C], BF16, tag="xb_all", name="xb_all")
        # alpha rings: h0 at [0:160], h1 at [160:320]
        al_ring = [sb.tile([128, 320], BF16, tag=f"al{j}", name=f"al{j}")
                   for j in range(4)]
        nc.vector.memset(v_tok[16][:, :], 0.0)
        nc.vector.memset(ones_r[:], 1.0)
        make_identity(nc, ident[:])

        # ---- constants / weights in ----
        nc.sync.dma_start(band[:], bandd)
        nc.sync.dma_start(bq_t[:], bqd.rearrange("(a b) -> b a", b=128))
        nc.sync.dma_start(bk_t[:], bkvd[0:C].rearrange("(a b) -> b a", b=128))
        nc.sync.dma_start(bvB[:], bkvd[C:2 * C][None, :].broadcast_to((128, C)))
        nc.sync.dma_start(bpB[:], bpd[None, :].broadcast_to((128, C)))
        nc.sync.dma_start(mq[:], maskd[PAD:PAD + NTOK].rearrange("(a b) -> b a", b=128))
        nc.sync.dma_start(mask_t[:, 0:16],
                          maskd[0:2048].rearrange("(a b) -> b a", b=128))
        nc.sync.dma_start(mask_t[0:32, 16:17], maskd[2048:2080][:, None])
        # ---- x in: 4 chunked loads, mask*cast per tile, one batched XBAR
        # transpose per 128-token tile ----
        xT3 = xT[:].rearrange("p (c r) -> p c r", c=4)
        xf3 = xf_all[:].rearrange("p (t c) -> p t c", t=17)
        xb3 = xb_all[:].rearrange("p (t c) -> p t c", t=17)
        xsrc = xd[0:2048, :].rearrange("(t p) c -> p t c", p=128)
        nc.sync.dma_start(xf3[:, 0:2], xsrc[:, 0:2])
        for t0, t1 in ((2, 7), (7, 12), (12, 16)):
            nc.scalar.dma_start(xf3[:, t0:t1], xsrc[:, t0:t1])
        nc.sync.dma_start(xf3[0:32, 16], xd[2048:2080, :])
        for t in range(17):
            r0, r1 = t * 128, min((t + 1) * 128, NKV)
            rows = r1 - r0
            nc.vector.tensor_scalar_mul(xb3[:rows, t], xf3[:rows, t],
                                        mask_t[:rows, t:t + 1])
            nc.scalar.dma_start_transpose(xT3[:, :, r0:r1], xb3[:rows, t])

        for ci in range(4):
            wqf = sb_x.tile([128, C], F32, tag="wld")
            nc.sync.dma_start(wqf[:], wqd[ci * 128:(ci + 1) * 128, :])
            nc.vector.tensor_copy(wq[ci][:], wqf[:])
            wkf = sb_x.tile([128, 2 * C], F32, tag="wld2")
            nc.sync.dma_start(wkf[:], wkvd[ci * 128:(ci + 1) * 128, :])
            nc.vector.tensor_copy(wk[ci][:], wkf[:, 0:C])
            nc.vector.tensor_copy(wv[ci][:], wkf[:, C:2 * C])
            wpf = sb_x.tile([128, C], F32, tag="wld")
            nc.sync.dma_start(wpf[:], wpd[ci * 128:(ci + 1) * 128, :])
            nc.vector.tensor_copy(wp[ci][:], wpf[:])
        nc.vector.tensor_copy(bp_row[:], bpB[0:1, :])

        # ---- qT (feature-major): W stationary, xT moving ----
        for co in range(4):
            for ch in range(4):
                t0 = ch * 512
                ps = pp_big.tile([128, 512], F32, tag="big")
                for ci in range(4):
                    nc.tensor.matmul(
                        ps[:], wq[ci][:, co * 128:(co + 1) * 128],
                        xT[:, ci * NKV + PAD + t0:ci * NKV + PAD + t0 + 512],
                        start=(ci == 0), stop=(ci == 3))
                nc.scalar.activation(qT[co][:, t0:t0 + 512], ps[:],
                                     mybir.ActivationFunctionType.Identity,
                                     bias=bq_t[:, co:co + 1])

        # ---- kT (feature-major) ----
        for co in range(4):
            t0 = 0
            for w in KCH:
                ps = pp_big.tile([128, 512], F32, tag="big")
                for ci in range(4):
                    nc.tensor.matmul(
                        ps[:, 0:w], wk[ci][:, co * 128:(co + 1) * 128],
                        xT[:, ci * NKV + t0:ci * NKV + t0 + w],
                        start=(ci == 0), stop=(ci == 3))
                nc.scalar.activation(kT[co][:, t0:t0 + w], ps[:, 0:w],
                                     mybir.ActivationFunctionType.Identity,
                                     bias=bk_t[:, co:co + 1])
                t0 += w

        # ---- v_tok (token-major): xT stationary, Wv moving ----
        def emit_v(t):
            r0, r1 = t * 128, min((t + 1) * 128, NKV)
            rows = r1 - r0
            ps = pp_big.tile([128, 512], F32, tag="big")
            for ci in range(4):
                nc.tensor.matmul(
                    ps[:rows, :], xT[:, ci * NKV + r0:ci * NKV + r1],
                    wv[ci][:], start=(ci == 0), stop=(ci == 3))
            nc.vector.scalar_tensor_tensor(
                v_tok[t][:rows, :], ps[:rows, :], 1.0, bvB[:rows, :],
                op0=MULT, op1=mybir.AluOpType.add)

        for t in range(17):
            emit_v(t)

        # ---- attention: software-pipelined over units u = (block, head-pair) ----
        NU = NB * 4
        state = {}

        def emit_scores(u):
            i, hp = divmod(u, 4)
            al = al_ring[u % 4]
            den = sb_den.tile([128, 2], F32, tag="den")
            rden = sb_den.tile([128, 2], F32, tag="rden")
            for hh in range(2):
                sc = pp_sc.tile([128, 160], F32, tag="sc", name="sc")
                nc.tensor.matmul(
                    sc[:],
                    qT[hp][hh * 64:(hh + 1) * 64, i * 128:(i + 1) * 128],
                    kT[hp][hh * 64:(hh + 1) * 64, i * 128:i * 128 + 160],
                    start=True, stop=True)
                a = al[:, hh * 160:hh * 160 + 160]
                nc.scalar.activation(a, sc[:],
                                     mybir.ActivationFunctionType.Exp,
                                     scale=SCALE)
                nc.vector.scalar_tensor_tensor(
                    a, a, 1.0, band[:], op0=MULT, op1=MULT,
                    accum_out=den[:, hh:hh + 1])
            nc.vector.reciprocal(rden[:], den[:])
            for hh in range(2):
                a = al[:, hh * 160:hh * 160 + 160]
                nc.vector.tensor_scalar_mul(a, a, rden[:, hh:hh + 1])
            state[u] = al

        def emit_transpose(u):
            al = state.pop(u)
            alT = sb_alt.tile([128, 512], BF16, tag="alT")
            for hh in range(2):
                tp = pp_tp.tile([128, 128], BF16, tag="tp", name="tp")
                nc.tensor.transpose(tp[:], al[:, hh * 160:hh * 160 + 128],
                                    ident[:])
                tl = pp_tp.tile([32, 128], BF16, tag="tp", name="tl")
                nc.tensor.transpose(tl[:], al[:, hh * 160 + 128:hh * 160 + 160],
                                    ident[:])
                if hh == 0:
                    nc.scalar.activation(alT[:, 0:128], tp[:],
                                         mybir.ActivationFunctionType.Copy)
                else:
                    nc.vector.tensor_copy(alT[:, 256:384], tp[:])
                if hh == 0:
                    nc.scalar.activation(alT[0:32, 128:256], tl[:],
                                         mybir.ActivationFunctionType.Copy)
                else:
                    nc.vector.tensor_copy(alT[0:32, 384:512], tl[:])
            state[("alT", u)] = alT

        def emit_av(u):
            i, hp = divmod(u, 4)
            alT = state.pop(("alT", u))
            at = pp_at.tile([128, 128], F32, tag="at", name="at")
            for hh in range(2):
                vc = hp * 128 + hh * 64
                nc.tensor.matmul(at[hh * 64:(hh + 1) * 64, :],
                                 v_tok[i][:, vc:vc + 64],
                                 alT[:, hh * 256:hh * 256 + 128],
                                 start=True, stop=False)
                nc.tensor.matmul(at[hh * 64:(hh + 1) * 64, :],
                                 v_tok[i + 1][0:32, vc:vc + 64],
                                 alT[0:32, hh * 256 + 128:hh * 256 + 256],
                                 start=False, stop=True)
            nc.scalar.activation(aT[:, hp * NTOK + i * 128:hp * NTOK + (i + 1) * 128],
                                 at[:], mybir.ActivationFunctionType.Copy)

        def emit_proj(i):
            ps = pp_big.tile([128, 512], F32, tag="big")
            for ci in range(4):
                nc.tensor.matmul(
                    ps[:], aT[:, ci * NTOK + i * 128:ci * NTOK + (i + 1) * 128],
                    wp[ci][:], start=(ci == 0), stop=False)
            nc.tensor.matmul(ps[:], ones_r[:], bp_row[:],
                             start=False, stop=True)
            ot = sb_o.tile([128, C], F32, tag="ot")
            nc.vector.tensor_scalar_mul(ot[:], ps[:], mq[:, i:i + 1])
            nc.sync.dma_start(outd[i * 128:(i + 1) * 128, :], ot[:])

        import os
        _NU = int(os.environ.get("KNU", str(NU)))
        _PH = int(os.environ.get("KPH", "3"))
        if _PH < 2:
            _NU = 0
        for u in range(_NU + 3):
            if u < _NU:
                emit_scores(u)
            if 2 <= u < _NU + 2:
                emit_transpose(u - 2)
            if u >= 3:
                emit_av(u - 3)
                if _PH >= 3 and (u - 3) % 4 == 3:
                    emit_proj((u - 3) // 4)
        if _PH < 3:
            dbg = sb.tile([128, C], F32, tag="dbg")
            nc.vector.tensor_copy(dbg[:], v_tok[0][:])
            nc.sync.dma_start(outd[0:128, :], dbg[:])

    nc.compile()
    return nc


_CACHE: dict = {}


def _get_program() -> bacc.Bacc:
    if "nc" not in _CACHE:
        _CACHE["nc"] = build_program()
    return _CACHE["nc"]


def kernel(x, mask, Wq, bq, Wkv, bkv, Wproj, bproj) -> np.ndarray:
    x = np.asarray(x, np.float32)
    mask = np.asarray(mask, np.float32)
    band = np.ascontiguousarray(_band_mask())
    nc = _get_program()

    in_maps = []
    for core in range(8):
        b, h = divmod(core, 2)
        s = h * NTOK
        xc = np.zeros((NKV, C), np.float32)
        mc = np.zeros((NKV,), np.float32)
        lo, hi = max(0, s - PAD), min(T, s + NTOK + PAD)
        xc[lo - (s - PAD):lo - (s - PAD) + hi - lo] = x[b, lo:hi]
        mc[lo - (s - PAD):lo - (s - PAD) + hi - lo] = mask[b, lo:hi]
        in_maps.append({
            "x": xc, "mask": mc,
            "wq": np.asarray(Wq, np.float32), "bq": np.asarray(bq, np.float32),
            "wkv": np.asarray(Wkv, np.float32), "bkv": np.asarray(bkv, np.float32),
            "wproj": np.asarray(Wproj, np.float32),
            "bproj": np.asarray(bproj, np.float32),
            "band": band,
        })

    res = bass_utils.run_bass_kernel_spmd(nc, in_maps, core_ids=list(range(8)))
    out = np.empty((B, T, C), np.float32)
    for core in range(8):
        b, h = divmod(core, 2)
        out[b, h * NTOK:(h + 1) * NTOK] = res.results[core]["out"]
    return out


# revision 21
# speedup vs baseline: 1.2638x; 1.2638x over previous
"""Local (sliding-window) self-attention Bass kernel for 8 TRN2 NeuronCores.

Problem: B=4, T=4096, C=512, H=8 heads, head_dim=64, window=15.
Sharding: 8 cores = batch(4) x seq-halves(2). Each core processes 2048 query
tokens of one batch element; its x chunk carries a 7-token halo on each side
(zero-padded at sequence edges, matching the reference's jnp.pad semantics),
padded to 2080 rows for 128/32 alignment.

Per-core dataflow (bf16 matmuls, fp32 PSUM accumulation):
  x chunk --mask*cast--> xb bf16 --one batched XBAR DMA transpose/tile--> xT_all
  qT = Wq-stationary matmuls + bias (feature-major, per head-pair tiles)
  kT = Wkv[:, :C]-stationary matmuls + bias (feature-major)
  v_tok = xT-stationary matmuls + bias (token-major)
  attention, software-pipelined over (block, head-pair) units:
    scores for both heads of a pair -> one [128,320] PSUM tile
    one exp activation (3D AP) -> al ring [128,512] with zeroed gaps
    per head on DVE: band-mask+den accum (STT), recip (paired), normalize (TS)
    alpha transpose on the PE array (4x [128,128] -> one [128,512] bf16 PSUM)
    one scalar copy -> alT SBUF; AV matmuls (K=128 incl. zero tail rows)
  out = attnT-stationary proj matmuls, (ps + bproj) * mask on DVE, DMA out
"""

import math
from contextlib import ExitStack

import ml_dtypes
import numpy as np

import concourse.bacc as bacc
import concourse.bass as bass
import concourse.mybir as mybir
import concourse.tile as tile
from concourse import bass_utils
from concourse.masks import make_identity

B, T, C, H, WIN = 4, 4096, 512, 8, 15
D = C // H            # 64
PAD = WIN // 2        # 7
NTOK = T // 2         # 2048 query tokens per core
NKV = 2080            # kv rows per core: 7 + 2048 + 7 = 2062, padded to 2080
NB = NTOK // 128      # 16 query blocks
KCH = [512, 512, 512, 512, 32]  # kv token chunks for feature-major matmuls
SCALE = math.log(WIN) / D
F32 = mybir.dt.float32
BF16 = mybir.dt.bfloat16
MULT = mybir.AluOpType.mult


def _band_mask() -> np.ndarray:
    """[128,160] multiplicative band: band[p, j] = 1 iff p <= j <= p+14."""
    p = np.arange(128)[:, None]
    j = np.arange(160)[None, :]
    return ((j >= p) & (j <= p + WIN - 1)).astype(ml_dtypes.bfloat16)


def build_program() -> bacc.Bacc:
    nc = bacc.Bacc("TRN2", target_bir_lowering=False, debug=False,
                   enable_asserts=False, num_devices=8)

    xd = nc.dram_tensor("x", [NKV, C], F32, kind="ExternalInput").ap()
    maskd = nc.dram_tensor("mask", [NKV], F32, kind="ExternalInput").ap()
    wqd = nc.dram_tensor("wq", [C, C], F32, kind="ExternalInput").ap()
    bqd = nc.dram_tensor("bq", [C], F32, kind="ExternalInput").ap()
    wkvd = nc.dram_tensor("wkv", [C, 2 * C], F32, kind="ExternalInput").ap()
    bkvd = nc.dram_tensor("bkv", [2 * C], F32, kind="ExternalInput").ap()
    wpd = nc.dram_tensor("wproj", [C, C], F32, kind="ExternalInput").ap()
    bpd = nc.dram_tensor("bproj", [C], F32, kind="ExternalInput").ap()
    bandd = nc.dram_tensor("band", [128, 160], BF16, kind="ExternalInput").ap()
    outd = nc.dram_tensor("out", [NTOK, C], F32, kind="ExternalOutput").ap()

    with tile.TileContext(nc) as tc, ExitStack() as ctx:
        sb = ctx.enter_context(tc.tile_pool(name="sb", bufs=1))
        sb_x = ctx.enter_context(tc.tile_pool(name="sb_x", bufs=3))
        sb_xin = ctx.enter_context(tc.tile_pool(name="sb_xin", bufs=8))
        sb_alt = ctx.enter_context(tc.tile_pool(name="sb_alt", bufs=3))
        sb_den = ctx.enter_context(tc.tile_pool(name="sb_den", bufs=4))
        sb_o = ctx.enter_context(tc.tile_pool(name="sb_o", bufs=3))
        pp_big = ctx.enter_context(tc.tile_pool(name="pp_big", bufs=1, space="PSUM"))
        pp_sc = ctx.enter_context(tc.tile_pool(name="pp_sc", bufs=2, space="PSUM"))
        pp_tp = ctx.enter_context(tc.tile_pool(name="pp_tp", bufs=4, space="PSUM"))
        pp_at = ctx.enter_context(tc.tile_pool(name="pp_at", bufs=1, space="PSUM"))

        # ---- persistent SBUF tensors ----
        xT = sb.tile([128, 4 * NKV], BF16, tag="xT", name="xT")
        qT = [sb.tile([128, NTOK], BF16, tag=f"qT{i}", name=f"qT{i}") for i in range(4)]
        kT = [sb.tile([128, NKV], BF16, tag=f"kT{i}", name=f"kT{i}") for i in range(4)]
        v_tok = [sb.tile([128, C], BF16, tag=f"vtok{i}", name=f"vtok{i}") for i in range(17)]
        aT = sb.tile([128, 4 * NTOK], BF16, tag="aT", name="aT")
        band = sb.tile([128, 160], BF16, tag="band")
        ident = sb.tile([128, 128], BF16, tag="ident")
        wq = [sb.tile([128, C], BF16, tag=f"wq{i}", name=f"wq{i}") for i in range(4)]
        wk = [sb.tile([128, C], BF16, tag=f"wk{i}", name=f"wk{i}") for i in range(4)]
        wv = [sb.tile([128, C], BF16, tag=f"wv{i}", name=f"wv{i}") for i in range(4)]
        wp = [sb.tile([128, C], BF16, tag=f"wp{i}", name=f"wp{i}") for i in range(4)]
        bq_t = sb.tile([128, 4], F32, tag="bq")       # per-partition q bias
        bk_t = sb.tile([128, 4], F32, tag="bk")       # per-partition k bias
        bvB = sb.tile([128, C], F32, tag="bvB")       # v bias bcast over partitions
        bpB = sb.tile([128, C], F32, tag="bpB")       # proj bias bcast
        mq = sb.tile([128, NB], F32, tag="mq")        # query-token mask, per block
        mask_t = sb.tile([128, 17], F32, tag="mask_t")  # mask, token-tiled
        ones_r = sb.tile([1, 128], BF16, tag="ones_r")
        bp_row = sb.tile([1, C], BF16, tag="bp_row")
        xf_all = sb.tile([128, 17 * C], F32, tag="xf_all", name="xf_all")
        xb_all = sb.tile([128, 17 * C], BF16, tag="xb_all", name="xb_all")
        # alpha rings: h0 at [0:160], h1 at [160:320]
        al_ring = [sb.tile([128, 320], BF16, tag=f"al{j}", name=f"al{j}")
                   for j in range(4)]
        nc.vector.memset(v_tok[16][:, :], 0.0)
        nc.vector.memset(ones_r[:], 1.0)
        make_identity(nc, ident[:])

        # ---- constants / weights in ----
        nc.sync.dma_start(band[:], bandd)
        nc.sync.dma_start(bq_t[:], bqd.rearrange("(a b) -> b a", b=128))
        nc.sync.dma_start(bk_t[:], bkvd[0:C].rearrange("(a b) -> b a", b=128))
        nc.sync.dma_start(bvB[:], bkvd[C:2 * C][None, :].broadcast_to((128, C)))
        nc.sync.dma_start(bpB[:], bpd[None, :].broadcast_to((128, C)))
        nc.sync.dma_start(mq[:], maskd[PAD:PAD + NTOK].rearrange("(a b) -> b a", b=128))
        nc.sync.dma_start(mask_t[:, 0:16],
                          maskd[0:2048].rearrange("(a b) -> b a", b=128))
        nc.sync.dma_start(mask_t[0:32, 16:17], maskd[2048:2080][:, None])
        # ---- x in: 4 chunked loads, mask*cast per tile, one batched XBAR
        # transpose per 128-token tile ----
        xT3 = xT[:].rearrange("p (c r) -> p c r", c=4)
        xf3 = xf_all[:].rearrange("p (t c) -> p t c", t=17)
        xb3 = xb_all[:].rearrange("p (t c) -> p t c", t=17)
        xsrc = xd[0:2048, :].rearrange("(t p) c -> p t c", p=128)
        for t0, t1 in ((0, 2), (2, 7), (7, 12), (12, 16)):
            nc.sync.dma_start(xf3[:, t0:t1], xsrc[:, t0:t1])
        nc.sync.dma_start(xf3[0:32, 16], xd[2048:2080, :])
        for t in range(17):
            r0, r1 = t * 128, min((t + 1) * 128, NKV)
            rows = r1 - r0
            nc.vector.tensor_scalar_mul(xb3[:rows, t], xf3[:rows, t],
                                        mask_t[:rows, t:t + 1])
            nc.scalar.dma_start_transpose(xT3[:, :, r0:r1], xb3[:rows, t])

        for ci in range(4):
            wqf = sb_x.tile([128, C], F32, tag="wld")
            nc.sync.dma_start(wqf[:], wqd[ci * 128:(ci + 1) * 128, :])
            nc.vector.tensor_copy(wq[ci][:], wqf[:])
            wkf = sb_x.tile([128, 2 * C], F32, tag="wld2")
            nc.sync.dma_start(wkf[:], wkvd[ci * 128:(ci + 1) * 128, :])
            nc.vector.tensor_copy(wk[ci][:], wkf[:, 0:C])
            nc.vector.tensor_copy(wv[ci][:], wkf[:, C:2 * C])
            wpf = sb_x.tile([128, C], F32, tag="wld")
            nc.sync.dma_start(wpf[:], wpd[ci * 128:(ci + 1) * 128, :])
            nc.vector.tensor_copy(wp[ci][:], wpf[:])
        nc.vector.tensor_copy(bp_row[:], bpB[0:1, :])

        # ---- qT (feature-major): W stationary, xT moving ----
        for co in range(4):
            for ch in range(4):
                t0 = ch * 512
                ps = pp_big.tile([128, 512], F32, tag="big")
                for ci in range(4):
                    nc.tensor.matmul(
                        ps[:], wq[ci][:, co * 128:(co + 1) * 128],
                        xT[:, ci * NKV + PAD + t0:ci * NKV + PAD + t0 + 512],
                        start=(ci == 0), stop=(ci == 3))
                nc.scalar.activation(qT[co][:, t0:t0 + 512], ps[:],
                                     mybir.ActivationFunctionType.Identity,
                                     bias=bq_t[:, co:co + 1])

        # ---- kT (feature-major) ----
        for co in range(4):
            t0 = 0
            for w in KCH:
                ps = pp_big.tile([128, 512], F32, tag="big")
                for ci in range(4):
                    nc.tensor.matmul(
                        ps[:, 0:w], wk[ci][:, co * 128:(co + 1) * 128],
                        xT[:, ci * NKV + t0:ci * NKV + t0 + w],
                        start=(ci == 0), stop=(ci == 3))
                nc.scalar.activation(kT[co][:, t0:t0 + w], ps[:, 0:w],
                                     mybir.ActivationFunctionType.Identity,
                                     bias=bk_t[:, co:co + 1])
                t0 += w

        # ---- v_tok (token-major): xT stationary, Wv moving ----
        def emit_v(t):
            r0, r1 = t * 128, min((t + 1) * 128, NKV)
            rows = r1 - r0
            ps = pp_big.tile([128, 512], F32, tag="big")
            for ci in range(4):
                nc.tensor.matmul(
                    ps[:rows, :], xT[:, ci * NKV + r0:ci * NKV + r1],
                    wv[ci][:], start=(ci == 0), stop=(ci == 3))
            nc.vector.scalar_tensor_tensor(
                v_tok[t][:rows, :], ps[:rows, :], 1.0, bvB[:rows, :],
                op0=MULT, op1=mybir.AluOpType.add)

        for t in range(17):
            emit_v(t)

        # ---- attention: software-pipelined over units u = (block, head-pair) ----
        NU = NB * 4
        state = {}

        def emit_scores(u):
            i, hp = divmod(u, 4)
            al = al_ring[u % 4]
            den = sb_den.tile([128, 2], F32, tag="den")
            rden = sb_den.tile([128, 2], F32, tag="rden")
            for hh in range(2):
                sc = pp_sc.tile([128, 160], F32, tag="sc", name="sc")
                nc.tensor.matmul(
                    sc[:],
                    qT[hp][hh * 64:(hh + 1) * 64, i * 128:(i + 1) * 128],
                    kT[hp][hh * 64:(hh + 1) * 64, i * 128:i * 128 + 160],
                    start=True, stop=True)
                a = al[:, hh * 160:hh * 160 + 160]
                nc.scalar.activation(a, sc[:],
                                     mybir.ActivationFunctionType.Exp,
                                     scale=SCALE)
                nc.vector.scalar_tensor_tensor(
                    a, a, 1.0, band[:], op0=MULT, op1=MULT,
                    accum_out=den[:, hh:hh + 1])
            nc.vector.reciprocal(rden[:], den[:])
            for hh in range(2):
                a = al[:, hh * 160:hh * 160 + 160]
                nc.vector.tensor_scalar_mul(a, a, rden[:, hh:hh + 1])
            state[u] = al

        def emit_transpose(u):
            al = state.pop(u)
            alT = sb_alt.tile([128, 512], BF16, tag="alT")
            for hh in range(2):
                tp = pp_tp.tile([128, 128], BF16, tag="tp", name="tp")
                nc.tensor.transpose(tp[:], al[:, hh * 160:hh * 160 + 128],
                                    ident[:])
                tl = pp_tp.tile([32, 128], BF16, tag="tp", name="tl")
                nc.tensor.transpose(tl[:], al[:, hh * 160 + 128:hh * 160 + 160],
                                    ident[:])
                if hh == 0:
                    nc.scalar.activation(alT[:, 0:128], tp[:],
                                         mybir.ActivationFunctionType.Copy)
                else:
                    nc.vector.tensor_copy(alT[:, 256:384], tp[:])
                nc.vector.tensor_copy(alT[0:32, hh * 256 + 128:hh * 256 + 256],
                                      tl[:])
            state[("alT", u)] = alT

        def emit_av(u):
            i, hp = divmod(u, 4)
            alT = state.pop(("alT", u))
            at = pp_at.tile([128, 128], F32, tag="at", name="at")
            for hh in range(2):
                vc = hp * 128 + hh * 64
                nc.tensor.matmul(at[hh * 64:(hh + 1) * 64, :],
                                 v_tok[i][:, vc:vc + 64],
                                 alT[:, hh * 256:hh * 256 + 128],
                                 start=True, stop=False)
                nc.tensor.matmul(at[hh * 64:(hh + 1) * 64, :],
                                 v_tok[i + 1][0:32, vc:vc + 64],
                                 alT[0:32, hh * 256 + 128:hh * 256 + 256],
                                 start=False, stop=True)
            nc.scalar.activation(aT[:, hp * NTOK + i * 128:hp * NTOK + (i + 1) * 128],
                                 at[:], mybir.ActivationFunctionType.Copy)

        def emit_proj(i):
            ps = pp_big.tile([128, 512], F32, tag="big")
            for ci in range(4):
                nc.tensor.matmul(
                    ps[:], aT[:, ci * NTOK + i * 128:ci * NTOK + (i + 1) * 128],
                    wp[ci][:], start=(ci == 0), stop=(ci == 3))
            ot = sb_o.tile([128, C], F32, tag="ot")
            nc.vector.tensor_tensor(ot[:], ps[:], bpB[:],
                                    op=mybir.AluOpType.add)
            nc.vector.tensor_scalar_mul(ot[:], ot[:], mq[:, i:i + 1])
            nc.sync.dma_start(outd[i * 128:(i + 1) * 128, :], ot[:])

        import os
        _NU = int(os.environ.get("KNU", str(NU)))
        _PH = int(os.environ.get("KPH", "3"))
        if _PH < 2:
            _NU = 0
        for u in range(_NU + 3):
            if u < _NU:
                emit_scores(u)
            if 2 <= u < _NU + 2:
                emit_transpose(u - 2)
            if u >= 3:
                emit_av(u - 3)
                if _PH >= 3 and (u - 3) % 4 == 3:
                    emit_proj((u - 3) // 4)
        if _PH < 3:
            dbg = sb.tile([128, C], F32, tag="dbg")
            nc.vector.tensor_copy(dbg[:], v_tok[0][:])
            nc.sync.dma_start(outd[0:128, :], dbg[:])

    nc.compile()
    return nc


_CACHE: dict = {}


def _get_program() -> bacc.Bacc:
    if "nc" not in _CACHE:
        _CACHE["nc"] = build_program()
    return _CACHE["nc"]


def kernel(x, mask, Wq, bq, Wkv, bkv, Wproj, bproj) -> np.ndarray:
    x = np.asarray(x, np.float32)
    mask = np.asarray(mask, np.float32)
    band = np.ascontiguousarray(_band_mask())
    nc = _get_program()

    in_maps = []
    for core in range(8):
        b, h = divmod(core, 2)
        s = h * NTOK
        xc = np.zeros((NKV, C), np.float32)
        mc = np.zeros((NKV,), np.float32)
        lo, hi = max(0, s - PAD), min(T, s + NTOK + PAD)
        xc[lo - (s - PAD):lo - (s - PAD) + hi - lo] = x[b, lo:hi]
        mc[lo - (s - PAD):lo - (s - PAD) + hi - lo] = mask[b, lo:hi]
        in_maps.append({
            "x": xc, "mask": mc,
            "wq": np.asarray(Wq, np.float32), "bq": np.asarray(bq, np.float32),
            "wkv": np.asarray(Wkv, np.float32), "bkv": np.asarray(bkv, np.float32),
            "wproj": np.asarray(Wproj, np.float32),
            "bproj": np.asarray(bproj, np.float32),
            "band": band,
        })

    res = bass_utils.run_bass_kernel_spmd(nc, in_maps, core_ids=list(range(8)))
    out = np.empty((B, T, C), np.float32)
    for core in range(8):
        b, h = divmod(core, 2)
        out[b, h * NTOK:(h + 1) * NTOK] = res.results[core]["out"]
    return out


# revision 30
# speedup vs baseline: 1.5786x; 1.2491x over previous
"""Local (sliding-window) self-attention Bass kernel for 8 TRN2 NeuronCores.

Problem: B=4, T=4096, C=512, H=8 heads, head_dim=64, window=15.
Sharding: 8 cores = batch(4) x seq-halves(2). Each core processes 2048 query
tokens of one batch element; its x chunk carries a 7-token halo on each side
(zero-padded at sequence edges, matching the reference's jnp.pad semantics),
padded to 2080 rows for 128/32 alignment.

Per-core dataflow (bf16 matmuls, fp32 PSUM accumulation), fully software-
pipelined so DMA transfers, GEMMs and attention overlap:
  x: 5 chunked DMA loads -> mask*cast (DVE) -> one batched 3D XBAR DMA
     transpose per 128-token tile into feature-major xT
  qT/kT: W-stationary matmuls + bias (feature-major); v: xT-stationary
     matmuls + bias (token-major); emitted chunk-wise between x loads and
     as background items inside the attention loop
  attention, 3-stage pipeline over units u = (block i, head-pair hp):
    scores [128q,160kv] per head (PSUM) -> exp on ACT -> band-mask*den
    accum (DVE STT) -> recip -> normalize (DVE) into alpha ring
    alpha transposed on the PE array (2 matmuls/head incl. 32-wide tail)
    -> PSUM->SBUF copies (ACT+DVE) -> AV matmuls accumulate [128,128]
    -> copy to aT (ACT)
  proj: aT-stationary matmuls; (ps + bproj) * mask on DVE -> DMA out
"""

import math
from contextlib import ExitStack

import ml_dtypes
import numpy as np

import concourse.bacc as bacc
import concourse.bass as bass
import concourse.mybir as mybir
import concourse.tile as tile
from concourse import bass_utils
from concourse.masks import make_identity

B, T, C, H, WIN = 4, 4096, 512, 8, 15
D = C // H            # 64
PAD = WIN // 2        # 7
NTOK = T // 2         # 2048 query tokens per core
NKV = 2080            # kv rows per core: 7 + 2048 + 7 = 2062, padded to 2080
NB = NTOK // 128      # 16 query blocks
KCH = [512, 512, 512, 512, 32]  # kv token chunks for feature-major matmuls
SCALE = math.log(WIN) / D
F32 = mybir.dt.float32
BF16 = mybir.dt.bfloat16
MULT = mybir.AluOpType.mult


def _band_mask() -> np.ndarray:
    """[128,160] multiplicative band: band[p, j] = 1 iff p <= j <= p+14."""
    p = np.arange(128)[:, None]
    j = np.arange(160)[None, :]
    return ((j >= p) & (j <= p + WIN - 1)).astype(ml_dtypes.bfloat16)


def build_program() -> bacc.Bacc:
    nc = bacc.Bacc("TRN2", target_bir_lowering=False, debug=False,
                   enable_asserts=False, num_devices=8)

    xd = nc.dram_tensor("x", [NKV, C], F32, kind="ExternalInput").ap()
    maskd = nc.dram_tensor("mask", [NKV], F32, kind="ExternalInput").ap()
    wqd = nc.dram_tensor("wq", [C, C], F32, kind="ExternalInput").ap()
    bqd = nc.dram_tensor("bq", [C], F32, kind="ExternalInput").ap()
    wkvd = nc.dram_tensor("wkv", [C, 2 * C], F32, kind="ExternalInput").ap()
    bkvd = nc.dram_tensor("bkv", [2 * C], F32, kind="ExternalInput").ap()
    wpd = nc.dram_tensor("wproj", [C, C], F32, kind="ExternalInput").ap()
    bpd = nc.dram_tensor("bproj", [C], F32, kind="ExternalInput").ap()
    bandd = nc.dram_tensor("band", [128, 160], BF16, kind="ExternalInput").ap()
    outd = nc.dram_tensor("out", [NTOK, C], F32, kind="ExternalOutput").ap()

    with tile.TileContext(nc) as tc, ExitStack() as ctx:
        sb = ctx.enter_context(tc.tile_pool(name="sb", bufs=1))
        sb_x = ctx.enter_context(tc.tile_pool(name="sb_x", bufs=3))
        sb_alt = ctx.enter_context(tc.tile_pool(name="sb_alt", bufs=3))
        sb_den = ctx.enter_context(tc.tile_pool(name="sb_den", bufs=4))
        sb_o = ctx.enter_context(tc.tile_pool(name="sb_o", bufs=3))
        pp_big = ctx.enter_context(tc.tile_pool(name="pp_big", bufs=1, space="PSUM"))
        pp_sc = ctx.enter_context(tc.tile_pool(name="pp_sc", bufs=2, space="PSUM"))
        pp_tp = ctx.enter_context(tc.tile_pool(name="pp_tp", bufs=4, space="PSUM"))
        pp_at = ctx.enter_context(tc.tile_pool(name="pp_at", bufs=1, space="PSUM"))

        # ---- persistent SBUF tensors ----
        xT = sb.tile([128, 4 * NKV], BF16, tag="xT", name="xT")
        qT = [sb.tile([128, NTOK], BF16, tag=f"qT{i}", name=f"qT{i}") for i in range(4)]
        kT = [sb.tile([128, NKV], BF16, tag=f"kT{i}", name=f"kT{i}") for i in range(4)]
        v_tok = [sb.tile([128, C], BF16, tag=f"vtok{i}", name=f"vtok{i}") for i in range(17)]
        aT = sb.tile([128, 4 * NTOK], BF16, tag="aT", name="aT")
        band = sb.tile([128, 160], BF16, tag="band")
        ident = sb.tile([128, 128], BF16, tag="ident")
        wq = [sb.tile([128, C], BF16, tag=f"wq{i}", name=f"wq{i}") for i in range(4)]
        wk = [sb.tile([128, C], BF16, tag=f"wk{i}", name=f"wk{i}") for i in range(4)]
        wv = [sb.tile([128, C], BF16, tag=f"wv{i}", name=f"wv{i}") for i in range(4)]
        wp = [sb.tile([128, C], BF16, tag=f"wp{i}", name=f"wp{i}") for i in range(4)]
        bq_t = sb.tile([128, 4], F32, tag="bq")       # per-partition q bias
        bk_t = sb.tile([128, 4], F32, tag="bk")       # per-partition k bias
        bvB = sb.tile([128, C], F32, tag="bvB")       # v bias bcast over partitions
        bpB = sb.tile([128, C], F32, tag="bpB")       # proj bias bcast
        mq = sb.tile([128, NB], F32, tag="mq")        # query-token mask, per block
        mask_t = sb.tile([128, 17], F32, tag="mask_t")  # mask, token-tiled
        xf_all = sb.tile([128, 17 * C], F32, tag="xf_all", name="xf_all")
        xb_all = sb.tile([128, 17 * C], BF16, tag="xb_all", name="xb_all")
        # alpha rings: h0 at [0:160], h1 at [160:320]
        al_ring = [sb.tile([128, 320], BF16, tag=f"al{j}", name=f"al{j}")
                   for j in range(4)]
        nc.vector.memset(v_tok[16][:, :], 0.0)
        make_identity(nc, ident[:])

        # ---- constants / weights in ----
        nc.sync.dma_start(mask_t[:, 0:16],
                          maskd[0:2048].rearrange("(a b) -> b a", b=128))
        nc.sync.dma_start(mask_t[0:32, 16:17], maskd[2048:2080][:, None])
        nc.gpsimd.dma_start(band[:], bandd)
        nc.gpsimd.dma_start(bq_t[:], bqd.rearrange("(a b) -> b a", b=128))
        nc.gpsimd.dma_start(bk_t[:], bkvd[0:C].rearrange("(a b) -> b a", b=128))
        nc.gpsimd.dma_start(bvB[:], bkvd[C:2 * C][None, :].broadcast_to((128, C)))
        nc.gpsimd.dma_start(bpB[:], bpd[None, :].broadcast_to((128, C)))
        nc.gpsimd.dma_start(mq[:], maskd[PAD:PAD + NTOK].rearrange("(a b) -> b a", b=128))
        # ---- x in: 4 chunked loads, mask*cast per tile, one batched XBAR
        # transpose per 128-token tile ----
        xT3 = xT[:].rearrange("p (c r) -> p c r", c=4)
        xf3 = xf_all[:].rearrange("p (t c) -> p t c", t=17)
        xb3 = xb_all[:].rearrange("p (t c) -> p t c", t=17)
        xsrc = xd[0:2048, :].rearrange("(t p) c -> p t c", p=128)
        for k, (t0, t1) in enumerate(
                ((0, 2), (2, 4), (4, 6), (6, 10), (10, 14), (14, 16))):
            eng = nc.sync if k % 2 == 0 else nc.gpsimd
            eng.dma_start(xf3[:, t0:t1], xsrc[:, t0:t1])
        nc.gpsimd.dma_start(xf3[0:32, 16], xd[2048:2080, :])
        for t in range(17):
            r0, r1 = t * 128, min((t + 1) * 128, NKV)
            rows = r1 - r0
            nc.vector.tensor_scalar_mul(xb3[:rows, t], xf3[:rows, t],
                                        mask_t[:rows, t:t + 1])
            nc.scalar.dma_start_transpose(xT3[:, :, r0:r1], xb3[:rows, t])

        for ci in range(4):
            wqf = sb_x.tile([128, C], F32, tag="wld")
            nc.gpsimd.dma_start(wqf[:], wqd[ci * 128:(ci + 1) * 128, :])
            nc.vector.tensor_copy(wq[ci][:], wqf[:])
            wkf = sb_x.tile([128, 2 * C], F32, tag="wld2")
            nc.gpsimd.dma_start(wkf[:], wkvd[ci * 128:(ci + 1) * 128, :])
            nc.vector.tensor_copy(wk[ci][:], wkf[:, 0:C])
            nc.vector.tensor_copy(wv[ci][:], wkf[:, C:2 * C])
            wpf = sb_x.tile([128, C], F32, tag="wld")
            nc.gpsimd.dma_start(wpf[:], wpd[ci * 128:(ci + 1) * 128, :])
            nc.vector.tensor_copy(wp[ci][:], wpf[:])

        # ---- qT (feature-major): W stationary, xT moving ----
        for co in range(4):
            for ch in range(4):
                t0 = ch * 512
                ps = pp_big.tile([128, 512], F32, tag="big")
                for ci in range(4):
                    nc.tensor.matmul(
                        ps[:], wq[ci][:, co * 128:(co + 1) * 128],
                        xT[:, ci * NKV + PAD + t0:ci * NKV + PAD + t0 + 512],
                        start=(ci == 0), stop=(ci == 3))
                nc.scalar.activation(qT[co][:, t0:t0 + 512], ps[:],
                                     mybir.ActivationFunctionType.Identity,
                                     bias=bq_t[:, co:co + 1])

        # ---- kT (feature-major) ----
        for co in range(4):
            t0 = 0
            for w in KCH:
                ps = pp_big.tile([128, 512], F32, tag="big")
                for ci in range(4):
                    nc.tensor.matmul(
                        ps[:, 0:w], wk[ci][:, co * 128:(co + 1) * 128],
                        xT[:, ci * NKV + t0:ci * NKV + t0 + w],
                        start=(ci == 0), stop=(ci == 3))
                nc.scalar.activation(kT[co][:, t0:t0 + w], ps[:, 0:w],
                                     mybir.ActivationFunctionType.Identity,
                                     bias=bk_t[:, co:co + 1])
                t0 += w

        # ---- v_tok (token-major): xT stationary, Wv moving ----
        def emit_v(t):
            r0, r1 = t * 128, min((t + 1) * 128, NKV)
            rows = r1 - r0
            ps = pp_big.tile([128, 512], F32, tag="big")
            for ci in range(4):
                nc.tensor.matmul(
                    ps[:rows, :], xT[:, ci * NKV + r0:ci * NKV + r1],
                    wv[ci][:], start=(ci == 0), stop=(ci == 3))
            nc.vector.scalar_tensor_tensor(
                v_tok[t][:rows, :], ps[:rows, :], 1.0, bvB[:rows, :],
                op0=MULT, op1=mybir.AluOpType.add)

        for t in range(17):
            emit_v(t)

        # ---- attention: software-pipelined over units u = (block, head-pair) ----
        NU = NB * 4
        state = {}

        def emit_scores(u):
            i, hp = divmod(u, 4)
            al = al_ring[u % 4]
            den = sb_den.tile([128, 2], F32, tag="den")
            rden = sb_den.tile([128, 2], F32, tag="rden")
            for hh in range(2):
                sc = pp_sc.tile([128, 160], F32, tag="sc", name="sc")
                nc.tensor.matmul(
                    sc[:],
                    qT[hp][hh * 64:(hh + 1) * 64, i * 128:(i + 1) * 128],
                    kT[hp][hh * 64:(hh + 1) * 64, i * 128:i * 128 + 160],
                    start=True, stop=True)
                a = al[:, hh * 160:hh * 160 + 160]
                nc.scalar.activation(a, sc[:],
                                     mybir.ActivationFunctionType.Exp,
                                     scale=SCALE)
                nc.vector.scalar_tensor_tensor(
                    a, a, 1.0, band[:], op0=MULT, op1=MULT,
                    accum_out=den[:, hh:hh + 1])
            nc.vector.reciprocal(rden[:], den[:])
            for hh in range(2):
                a = al[:, hh * 160:hh * 160 + 160]
                nc.vector.tensor_scalar_mul(a, a, rden[:, hh:hh + 1])
            state[u] = al

        def emit_transpose(u):
            al = state.pop(u)
            alT = sb_alt.tile([128, 512], BF16, tag="alT")
            for hh in range(2):
                tp = pp_tp.tile([128, 128], BF16, tag="tp", name="tp")
                nc.tensor.transpose(tp[:], al[:, hh * 160:hh * 160 + 128],
                                    ident[:])
                tl = pp_tp.tile([32, 128], BF16, tag="tp", name="tl")
                nc.tensor.transpose(tl[:], al[:, hh * 160 + 128:hh * 160 + 160],
                                    ident[:])
                if hh == 0:
                    nc.scalar.activation(alT[:, 0:128], tp[:],
                                         mybir.ActivationFunctionType.Copy)
                else:
                    nc.vector.tensor_copy(alT[:, 256:384], tp[:])
                nc.vector.tensor_copy(alT[0:32, hh * 256 + 128:hh * 256 + 256],
                                      tl[:])
            state[("alT", u)] = alT

        def emit_av(u):
            i, hp = divmod(u, 4)
            alT = state.pop(("alT", u))
            at = pp_at.tile([128, 128], F32, tag="at", name="at")
            for hh in range(2):
                vc = hp * 128 + hh * 64
                nc.tensor.matmul(at[hh * 64:(hh + 1) * 64, :],
                                 v_tok[i][:, vc:vc + 64],
                                 alT[:, hh * 256:hh * 256 + 128],
                                 start=True, stop=False)
                nc.tensor.matmul(at[hh * 64:(hh + 1) * 64, :],
                                 v_tok[i + 1][0:32, vc:vc + 64],
                                 alT[0:32, hh * 256 + 128:hh * 256 + 256],
                                 start=False, stop=True)
            nc.scalar.activation(aT[:, hp * NTOK + i * 128:hp * NTOK + (i + 1) * 128],
                                 at[:], mybir.ActivationFunctionType.Copy)

        def emit_proj(i):
            ps = pp_big.tile([128, 512], F32, tag="big")
            for ci in range(4):
                nc.tensor.matmul(
                    ps[:], aT[:, ci * NTOK + i * 128:ci * NTOK + (i + 1) * 128],
                    wp[ci][:], start=(ci == 0), stop=(ci == 3))
            ot = sb_o.tile([128, C], F32, tag="ot")
            nc.vector.tensor_tensor(ot[:], ps[:], bpB[:],
                                    op=mybir.AluOpType.add)
            nc.vector.tensor_scalar_mul(ot[:], ot[:], mq[:, i:i + 1])
            nc.sync.dma_start(outd[i * 128:(i + 1) * 128, :], ot[:])

        import os
        _NU = int(os.environ.get("KNU", str(NU)))
        _PH = int(os.environ.get("KPH", "3"))
        if _PH < 2:
            _NU = 0
        for u in range(_NU + 3):
            if u < _NU:
                emit_scores(u)
            if 2 <= u < _NU + 2:
                emit_transpose(u - 2)
            if u >= 3:
                emit_av(u - 3)
                if _PH >= 3 and (u - 3) % 4 == 3:
                    emit_proj((u - 3) // 4)
        if _PH < 3:
            dbg = sb.tile([128, C], F32, tag="dbg")
            nc.vector.tensor_copy(dbg[:], v_tok[0][:])
            nc.sync.dma_start(outd[0:128, :], dbg[:])

    nc.compile()
    return nc


_CACHE: dict = {}


def _get_program() -> bacc.Bacc:
    if "nc" not in _CACHE:
        _CACHE["nc"] = build_program()
    return _CACHE["nc"]


def kernel(x, mask, Wq, bq, Wkv, bkv, Wproj, bproj) -> np.ndarray:
    x = np.asarray(x, np.float32)
    mask = np.asarray(mask, np.float32)
    band = np.ascontiguousarray(_band_mask())
    nc = _get_program()

    in_maps = []
    for core in range(8):
        b, h = divmod(core, 2)
        s = h * NTOK
        xc = np.zeros((NKV, C), np.float32)
        mc = np.zeros((NKV,), np.float32)
        lo, hi = max(0, s - PAD), min(T, s + NTOK + PAD)
        xc[lo - (s - PAD):lo - (s - PAD) + hi - lo] = x[b, lo:hi]
        mc[lo - (s - PAD):lo - (s - PAD) + hi - lo] = mask[b, lo:hi]
        in_maps.append({
            "x": xc, "mask": mc,
            "wq": np.asarray(Wq, np.float32), "bq": np.asarray(bq, np.float32),
            "wkv": np.asarray(Wkv, np.float32), "bkv": np.asarray(bkv, np.float32),
            "wproj": np.asarray(Wproj, np.float32),
            "bproj": np.asarray(bproj, np.float32),
            "band": band,
        })

    res = bass_utils.run_bass_kernel_spmd(nc, in_maps, core_ids=list(range(8)))
    out = np.empty((B, T, C), np.float32)
    for core in range(8):
        b, h = divmod(core, 2)
        out[b, h * NTOK:(h + 1) * NTOK] = res.results[core]["out"]
    return out
